# revision 41
# baseline (speedup 1.0000x reference)
"""Trainium2 Bass kernel for masked-pool + per-sample expert matmul (moe_routing).

Computation (reference):
    attended[b,c] = mean_hw(mask[b,hw] * features[b,c,hw])        # [B,C]
    preds[b,a]    = sum_c attended[b,c] * weight[inst[b],c,a] + bias[inst[b],a]

Sharding: expert-parallel with host-side routing. The 32 experts are packed
into 8 bins of 4 (balanced by sample count); each core gets the features of
the samples routed to its 4 experts, its 4 experts' weights, and an indicator
matrix ind[slot, row] marking which rows belong to which expert slot.

All large operands are staged in fp8-e3m4 (4 mantissa bits, ~1.25% rms
quantization noise; the end-to-end rel err lands ~1.2e-2 vs the 2e-2 gate).
Weights are pre-scaled by ~128 so their magnitudes sit in e3m4's normal range
(max ~13.9 < 15.5); the descale rides along in the bf16 indicator constant
(~1/(196*128), carrying the spatial-mean divisor too), and the host weight
scale compensates that constant's bf16 rounding exactly. This halves HBM
traffic again vs bf16: ~7.6MB/core (~21us at 360GB/s).

Phase 1 (attended) runs on the PE as per-sample matvecs: features staged with
the spatial dim on partitions ([98x2] split of hw=196), so
attT[c, s] = sum_hw feat[hw, c] * mask[hw, s] is a chain of two 1-column
matmuls per (sample, c-chunk) accumulating in PSUM; a DVE multiply folds the
per-expert indicator (carrying mean + descale) into a bf16 matt[c, g, s, j].

Phase 2 streams the fp8 weights answer-chunk-serially: each 256KB weight DMA
covers one 128-col answer chunk for all 4 experts (512B contiguous lines);
right behind it the PE runs one bias matmul (K=4, bias arriving via
be.T @ onehot) plus 16 accumulate matmuls (expert x c-chunk) into a single
psum bank, then a DVE copy drains the chunk to bf16 SBUF. Accumulation
chains are strictly sequential per bank so no interleaved-chain hazards
exist, and the natural program order on the single SP DMA queue is already
the optimal serial transfer order (features -> weights -> outputs) with no
gating needed. The final output DMA covers only the last four chunks so a
single ~1.3us DMA-issue pipeline sits on the drain tail.
"""

import numpy as np
import ml_dtypes

import concourse.bacc as bacc
import concourse.tile as tile
from concourse import mybir
from concourse.bass_utils import run_bass_kernel_spmd

BF16 = ml_dtypes.bfloat16
F8E3 = ml_dtypes.float8_e3m4

B, C, H, W = 256, 512, 14, 14
HWD = H * W  # 196
HW1 = HWD // 2  # 98 spatial positions per partition-chunk
UC = 2          # spatial chunks (2*98 = 196)
N_EXP, N_ANS = 32, 2000
N_CORES = 8
E = N_EXP // N_CORES  # expert slots per core = 4
S_DEFAULT = 32        # padded samples per core (>= max balanced bin load)
J = C // 128          # c-chunks = 4
A_PAD = 2048          # padded answer dim (2000 -> 16 chunks of 128)
NCH = A_PAD // 128    # 16 answer chunks of 128 cols = weight DMA granularity
GB = 16               # samples per feature-DMA batch
WSCALE = 128.0        # nominal weight pre-scale into e3m4 normal range
# the indicator is stored bf16; fold its rounding into the host weight scale
# so the device's matt * wq product is exactly w * att
K_IND = float(np.asarray(1.0 / (HWD * WSCALE), np.float32).astype(
    ml_dtypes.bfloat16))
W_DIV = float(np.float64(HWD) * np.float64(K_IND))  # wq = w / W_DIV

_compiled = {}  # S -> nc
_runners = {}   # S -> callable(in_maps) -> per-core result dicts


def _make_runner(nc):
    """Build a reusable jitted SPMD executor for `nc` (jit traced once, so
    repeat kernel() calls skip retracing; mirrors bass2jax.run_bass_via_pjrt).
    """
    import jax
    from jax.experimental.shard_map import shard_map
    from jax.sharding import Mesh, PartitionSpec
    from concourse.bass2jax import (_bass_exec_p, install_neuronx_cc_hook,
                                    partition_id_tensor)

    install_neuronx_cc_hook()
    pname = nc.partition_id_tensor.name if nc.partition_id_tensor else None
    in_names, out_names, out_avals = [], [], []
    for alloc in nc.m.functions[0].allocations:
        if not isinstance(alloc, mybir.MemoryLocationSet):
            continue
        name = alloc.memorylocations[0].name
        if alloc.kind == "ExternalInput":
            if name != pname:
                in_names.append(name)
        elif alloc.kind == "ExternalOutput":
            out_names.append(name)
            out_avals.append(jax.core.ShapedArray(
                tuple(alloc.tensor_shape), mybir.dt.np(alloc.dtype)))
    n_params = len(in_names)
    n_outs = len(out_avals)
    all_in = in_names + out_names + ([pname] if pname else [])
    donate = tuple(range(n_params, n_params + n_outs))

    def _body(*args):
        operands = list(args)
        if pname is not None:
            operands.append(partition_id_tensor())
        return tuple(_bass_exec_p.bind(
            *operands, out_avals=tuple(out_avals), in_names=tuple(all_in),
            out_names=tuple(out_names), lowering_input_output_aliases=(),
            sim_require_finite=True, sim_require_nnan=True, nc=nc))

    devices = jax.devices()[:N_CORES]
    mesh = Mesh(np.asarray(devices), ("core",))
    sharded = jax.jit(
        shard_map(_body, mesh=mesh,
                  in_specs=(PartitionSpec("core"),) * (n_params + n_outs),
                  out_specs=(PartitionSpec("core"),) * n_outs,
                  check_rep=False),
        donate_argnums=donate, keep_unused=True)

    def run(in_maps):
        concat_in = [
            np.concatenate([np.asarray(m[name]) for m in in_maps], axis=0)
            for name in in_names
        ]
        zeros = [np.zeros((N_CORES * a.shape[0], *a.shape[1:]), a.dtype)
                 for a in out_avals]
        out = sharded(*concat_in, *zeros)
        return [
            {name: np.asarray(out[i]).reshape(N_CORES, *out_avals[i].shape)[c]
             for i, name in enumerate(out_names)}
            for c in range(N_CORES)
        ]

    return run


def _get_runner(S):
    if S not in _runners:
        _runners[S] = _make_runner(_get_compiled(S))
    return _runners[S]


def _build(S):
    fp32 = mybir.dt.float32
    bf16 = mybir.dt.bfloat16
    f8 = mybir.dt.float8e3
    nc = bacc.Bacc("TRN2", target_bir_lowering=False, debug=False,
                   num_devices=N_CORES)
    # features with hw on partitions: feat[u, q, s, c] = x[samp_s, c, u*98+q]
    feat = nc.dram_tensor("feat", [UC, HW1, S, C], f8, kind="ExternalInput")
    maskq = nc.dram_tensor("maskq", [HW1, UC, S], bf16, kind="ExternalInput")
    # weights packed per 128-col chunk: wt[c, g, p, j, a] = wq[g, j*128+p, c*128+a]
    wt = nc.dram_tensor("wt", [NCH, E, 128, J, 128], f8, kind="ExternalInput")
    be = nc.dram_tensor("be", [E, A_PAD], bf16, kind="ExternalInput")
    ind = nc.dram_tensor("ind", [1, E, S], bf16, kind="ExternalInput")
    ind01 = nc.dram_tensor("ind01", [E, S], bf16, kind="ExternalInput")
    outT = nc.dram_tensor("outT", [A_PAD, S], bf16, kind="ExternalOutput")

    with tile.TileContext(nc) as tc:
        with (
            tc.tile_pool(name="persist", bufs=1) as persist,
            tc.tile_pool(name="fpool", bufs=2) as fpool,
            tc.tile_pool(name="wpool", bufs=10) as wpool,
            tc.tile_pool(name="psb", bufs=2, space="PSUM") as psb_pool,
            tc.tile_pool(name="ps2", bufs=2, space="PSUM") as ps2_pool,
        ):
            mask_sb = persist.tile([HW1, UC, S], bf16)
            indb = persist.tile([128, E, S], bf16)
            be_sb = persist.tile([E, A_PAD], bf16)
            i01_sb = persist.tile([E, S], bf16)

            # indicator-masked attended, bf16, for phase-2 streaming
            matt = persist.tile([128, E, S, J], bf16)
            out_sb = persist.tile([128, NCH, S], bf16)

            # phase 1: attT[c, s] = sum_hw feat[hw, c] * mask[hw, s] via PE
            # matvec chains (two 98-row matmuls per sample and c-chunk)
            first = True
            for i0 in range(0, S, GB):
                g_sz = min(GB, S - i0)
                ft = fpool.tile([HW1, UC, GB, C], f8, tag="ft")
                nc.sync.dma_start(
                    ft[:, :, :g_sz],
                    feat.ap()[:, :, i0:i0 + g_sz].rearrange(
                        "u q s c -> q u s c"))
                if first:
                    # small loads slot in right behind the first feature batch
                    nc.sync.dma_start(mask_sb[:], maskq.ap())
                    nc.sync.dma_start(indb[:],
                                      ind.ap().to_broadcast((128, E, S)))
                    nc.sync.dma_start(be_sb[:], be.ap())
                    nc.sync.dma_start(i01_sb[:], ind01.ap())
                    first = False
                ps_b = psb_pool.tile([128, GB, J], fp32, tag="psb")
                for s in range(g_sz):
                    for j in range(J):
                        for u in range(UC):
                            nc.tensor.matmul(
                                ps_b[:, s, j:j + 1],
                                ft[:, u, s, j * 128:(j + 1) * 128],
                                mask_sb[:, u, i0 + s:i0 + s + 1],
                                start=(u == 0), stop=(u == UC - 1))
                # fold the indicator (mean scale + weight descale) and cast
                # to bf16 for the phase-2 matmuls
                for g in range(E):
                    nc.vector.tensor_mul(
                        matt[:, g, i0:i0 + g_sz, :],
                        ps_b[:, :g_sz, :],
                        indb[:, g, i0:i0 + g_sz, None].to_broadcast(
                            (128, g_sz, J)))

            # phase 2, chunk-serial: psum[a, s] = be.T @ onehot
            #   + sum_g sum_j wt[g, j, a].T @ matt[:, g, :, j]
            for ac in range(NCH):
                wtile = wpool.tile([128, E, J, 128], f8, tag="wt")
                nc.sync.dma_start(wtile[:],
                                  wt.ap()[ac].rearrange("g p j a -> p g j a"))
                ps = ps2_pool.tile([128, 512], fp32, tag="ps2")
                nc.tensor.matmul(
                    ps[:, :S],
                    be_sb[:, ac * 128:(ac + 1) * 128],
                    i01_sb[:],
                    start=True, stop=False)
                for g in range(E):
                    for j in range(J):
                        nc.tensor.matmul(
                            ps[:, :S],
                            wtile[:, g, j, :],
                            matt[:, g, :, j],
                            start=False,
                            stop=(g == E - 1 and j == J - 1))
                nc.vector.tensor_copy(out_sb[:, ac, :], ps[:, :S])

            # output DMAs, streamed right behind the weight tail; the bulk
            # of the tail group goes out against chunk 14's copy, leaving
            # only a 128-row DMA dependent on the final chunk.
            for c0, c1 in ((0, 6), (6, 12), (12, 15), (15, 16)):
                nc.sync.dma_start(
                    outT.ap()[c0 * 128:c1 * 128].rearrange(
                        "(c p) s -> p c s", p=128),
                    out_sb[:, c0:c1, :])
    nc.compile()
    return nc


def _get_compiled(S):
    if S not in _compiled:
        _compiled[S] = _build(S)
    return _compiled[S]


def _exact_partition(cnt, cap):
    """Try to split the 32 experts into 8 groups of 4 with group-sum <= cap.

    Builds groups one at a time: each group takes the largest remaining
    expert plus 3 companions chosen by DFS over distinct count-combinations.
    Returns bins (list of expert-id groups) or None.
    """
    import itertools

    budget = [500000]

    def solve(ids):
        if not ids:
            return []
        if budget[0] <= 0:
            return None
        ids = sorted(ids, key=lambda e: -cnt[e])
        first = ids[0]
        rest = ids[1:]
        n = len(rest)
        seen = set()
        for combo in itertools.combinations(range(n), E - 1):
            budget[0] -= 1
            if budget[0] <= 0:
                return None
            vals = tuple(cnt[rest[i]] for i in combo)
            if cnt[first] + sum(vals) > cap or vals in seen:
                continue
            seen.add(vals)
            remaining = [rest[i] for i in range(n) if i not in combo]
            sub = solve(remaining)
            if sub is not None:
                return [[first] + [rest[i] for i in combo]] + sub
        return None

    return solve(list(range(N_EXP)))


def _route(instance):
    """Pack 32 experts into 8 bins of 4, balanced by sample count.

    Returns (bins, sample_lists, max_load): bins[c] = 4 expert ids,
    sample_lists[c] = sample indices routed to core c (grouped by expert).
    """
    cnt = np.bincount(instance, minlength=N_EXP)
    # perfect balance first: groups of 4 experts each with <= ceil(B/8)
    cap = (int(cnt.sum()) + N_CORES - 1) // N_CORES
    bins = _exact_partition(cnt, cap)
    if bins is None:
        order = np.argsort(-cnt, kind="stable")
        bins = [[] for _ in range(N_CORES)]
        loads = [0] * N_CORES
        for e in order:
            cands = [b for b in range(N_CORES) if len(bins[b]) < E]
            b = min(cands, key=lambda x: loads[x])
            bins[b].append(int(e))
            loads[b] += int(cnt[e])
    sample_lists = [
        np.concatenate([np.where(instance == e)[0] for e in bins[c]])
        for c in range(N_CORES)
    ]
    return bins, sample_lists, max(len(s) for s in sample_lists)


def make_in_maps(mask, features, weight, bias, inst, S, bins, sample_lists):
    feat_q = features.reshape(B, C, HWD).astype(F8E3)
    mask_flat = mask.reshape(B, HWD)
    # weights into e3m4 normal range; pad answer dim to A_PAD
    wq = (weight / W_DIV).astype(F8E3)
    in_maps = []
    for c in range(N_CORES):
        samp = sample_lists[c]
        n_c = len(samp)
        if n_c > 0:
            padded = np.concatenate([samp, np.full(S - n_c, samp[0])])
        else:
            padded = np.zeros(S, dtype=np.int64)
        ind_c = np.zeros((1, E, S), dtype=BF16)
        slot_of = {e: g for g, e in enumerate(bins[c])}
        for k in range(n_c):
            ind_c[0, slot_of[int(inst[samp[k]])], k] = K_IND
        be_c = np.zeros((E, A_PAD), dtype=BF16)
        be_c[:, :N_ANS] = bias[bins[c]].astype(BF16)
        ind01_c = (np.asarray(ind_c[0], np.float32) != 0).astype(BF16)
        # [S,C,HWD] -> [HWD,S,C] -> [2,98,S,C] with hw = u*98+q
        feat_c = np.ascontiguousarray(
            feat_q[padded].transpose(2, 0, 1)).reshape(UC, HW1, S, C)
        # mask packed [q, u, s] so the DMA needs no rearrange
        mask_c = np.ascontiguousarray(
            mask_flat[padded].astype(BF16).T.reshape(UC, HW1, S)
            .transpose(1, 0, 2))
        wq_c = np.zeros((E, C, A_PAD), dtype=F8E3)
        wq_c[:, :, :N_ANS] = wq[bins[c]]
        # wt[ac, g, p, j, a] = wq_c[g, j*128+p, ac*128+a]
        wt_c = np.ascontiguousarray(
            wq_c.reshape(E, J, 128, NCH, 128).transpose(3, 0, 2, 1, 4))
        in_maps.append({
            "feat": feat_c,
            "maskq": mask_c,
            "wt": wt_c,
            "be": be_c,
            "ind": ind_c,
            "ind01": ind01_c,
        })
    return in_maps


def kernel(mask, features, weight, bias, instance):
    mask = np.ascontiguousarray(np.asarray(mask, dtype=np.float32))
    features = np.ascontiguousarray(np.asarray(features, dtype=np.float32))
    weight = np.ascontiguousarray(np.asarray(weight, dtype=np.float32))
    bias = np.ascontiguousarray(np.asarray(bias, dtype=np.float32))
    inst = np.asarray(instance).astype(np.int64)
    assert features.shape == (B, C, H, W)

    bins, sample_lists, max_load = _route(inst)
    S = max(S_DEFAULT, max_load)
    nc = _get_compiled(S)

    in_maps = make_in_maps(mask, features, weight, bias, inst, S, bins,
                           sample_lists)
    try:
        results = _get_runner(S)(in_maps)
    except Exception:
        results = run_bass_kernel_spmd(
            nc, in_maps, list(range(N_CORES))).results

    preds = np.empty((B, N_ANS), dtype=np.float32)
    for c in range(N_CORES):
        samp = sample_lists[c]
        preds[samp] = results[c]["outT"][:N_ANS, :len(samp)].astype(
            np.float32).T
    return preds


# Precompile the default-size program at import so a timed first call does
# not pay the (one-time) build+compile cost.
_get_compiled(S_DEFAULT)


# revision 46
# speedup vs baseline: 1.0311x; 1.0311x over previous
"""Trainium2 Bass kernel for masked-pool + per-sample expert matmul (moe_routing).

Computation (reference):
    attended[b,c] = mean_hw(mask[b,hw] * features[b,c,hw])        # [B,C]
    preds[b,a]    = sum_c attended[b,c] * weight[inst[b],c,a] + bias[inst[b],a]

Sharding: expert-parallel with host-side routing. The 32 experts are packed
into 8 bins of 4 (balanced by sample count); each core gets the features of
the samples routed to its 4 experts, its 4 experts' weights, and an indicator
matrix ind[slot, row] marking which rows belong to which expert slot.

All large operands are staged in fp8-e3m4 (4 mantissa bits, ~1.25% rms
quantization noise; the end-to-end rel err lands ~1.2e-2 vs the 2e-2 gate).
Weights are pre-scaled by ~128 so their magnitudes sit in e3m4's normal range
(max ~13.9 < 15.5); the descale rides along in the bf16 indicator constant
(~1/(196*128), carrying the spatial-mean divisor too), and the host weight
scale compensates that constant's bf16 rounding exactly. This halves HBM
traffic again vs bf16: ~7.6MB/core (~21us at 360GB/s).

Phase 1 (attended) runs on the PE as per-sample matvecs: features staged with
the spatial dim on partitions ([98x2] split of hw=196), so
attT[c, s] = sum_hw feat[hw, c] * mask[hw, s] is a chain of two 1-column
matmuls per (sample, c-chunk) accumulating in PSUM; a DVE multiply folds the
per-expert indicator (carrying mean + descale) into a bf16 matt[c, g, s, j].

Phase 2 streams the fp8 weights answer-chunk-serially: each 256KB weight DMA
covers one 128-col answer chunk for all 4 experts (512B contiguous lines);
right behind it the PE runs one bias matmul (K=4, bias arriving via
be.T @ onehot) plus 16 accumulate matmuls (expert x c-chunk) into a single
psum bank, then a DVE copy drains the chunk to bf16 SBUF. Accumulation
chains are strictly sequential per bank so no interleaved-chain hazards
exist, and the natural program order on the single SP DMA queue is already
the optimal serial transfer order (features -> weights -> outputs) with no
gating needed. Outputs drain in three groups (bulk / late pair / final
chunk) with 4 psum bufs — the shape that measured best against the Tile
scheduler's sem-wait coalescing — so only a single ~1.3us DMA-issue
pipeline plus a 56ns transfer trail the last weight tile.
"""

import numpy as np
import ml_dtypes

import concourse.bacc as bacc
import concourse.tile as tile
from concourse import mybir
from concourse.bass_utils import run_bass_kernel_spmd

BF16 = ml_dtypes.bfloat16
F8E3 = ml_dtypes.float8_e3m4

B, C, H, W = 256, 512, 14, 14
HWD = H * W  # 196
HW1 = HWD // 2  # 98 spatial positions per partition-chunk
UC = 2          # spatial chunks (2*98 = 196)
N_EXP, N_ANS = 32, 2000
N_CORES = 8
E = N_EXP // N_CORES  # expert slots per core = 4
S_DEFAULT = 32        # padded samples per core (>= max balanced bin load)
J = C // 128          # c-chunks = 4
A_PAD = 2048          # padded answer dim (2000 -> 16 chunks of 128)
NCH = A_PAD // 128    # 16 answer chunks of 128 cols = weight DMA granularity
GB = 16               # samples per feature-DMA batch
WSCALE = 128.0        # nominal weight pre-scale into e3m4 normal range
# the indicator is stored bf16; fold its rounding into the host weight scale
# so the device's matt * wq product is exactly w * att
K_IND = float(np.asarray(1.0 / (HWD * WSCALE), np.float32).astype(
    ml_dtypes.bfloat16))
W_DIV = float(np.float64(HWD) * np.float64(K_IND))  # wq = w / W_DIV

_compiled = {}  # S -> nc
_runners = {}   # S -> callable(in_maps) -> per-core result dicts


def _make_runner(nc):
    """Build a reusable jitted SPMD executor for `nc` (jit traced once, so
    repeat kernel() calls skip retracing; mirrors bass2jax.run_bass_via_pjrt).
    """
    import jax
    from jax.experimental.shard_map import shard_map
    from jax.sharding import Mesh, PartitionSpec
    from concourse.bass2jax import (_bass_exec_p, install_neuronx_cc_hook,
                                    partition_id_tensor)

    install_neuronx_cc_hook()
    pname = nc.partition_id_tensor.name if nc.partition_id_tensor else None
    in_names, out_names, out_avals = [], [], []
    for alloc in nc.m.functions[0].allocations:
        if not isinstance(alloc, mybir.MemoryLocationSet):
            continue
        name = alloc.memorylocations[0].name
        if alloc.kind == "ExternalInput":
            if name != pname:
                in_names.append(name)
        elif alloc.kind == "ExternalOutput":
            out_names.append(name)
            out_avals.append(jax.core.ShapedArray(
                tuple(alloc.tensor_shape), mybir.dt.np(alloc.dtype)))
    n_params = len(in_names)
    n_outs = len(out_avals)
    all_in = in_names + out_names + ([pname] if pname else [])
    donate = tuple(range(n_params, n_params + n_outs))

    def _body(*args):
        operands = list(args)
        if pname is not None:
            operands.append(partition_id_tensor())
        return tuple(_bass_exec_p.bind(
            *operands, out_avals=tuple(out_avals), in_names=tuple(all_in),
            out_names=tuple(out_names), lowering_input_output_aliases=(),
            sim_require_finite=True, sim_require_nnan=True, nc=nc))

    devices = jax.devices()[:N_CORES]
    mesh = Mesh(np.asarray(devices), ("core",))
    sharded = jax.jit(
        shard_map(_body, mesh=mesh,
                  in_specs=(PartitionSpec("core"),) * (n_params + n_outs),
                  out_specs=(PartitionSpec("core"),) * n_outs,
                  check_rep=False),
        donate_argnums=donate, keep_unused=True)

    def run(in_maps):
        concat_in = [
            np.concatenate([np.asarray(m[name]) for m in in_maps], axis=0)
            for name in in_names
        ]
        zeros = [np.zeros((N_CORES * a.shape[0], *a.shape[1:]), a.dtype)
                 for a in out_avals]
        out = sharded(*concat_in, *zeros)
        return [
            {name: np.asarray(out[i]).reshape(N_CORES, *out_avals[i].shape)[c]
             for i, name in enumerate(out_names)}
            for c in range(N_CORES)
        ]

    return run


def _get_runner(S):
    if S not in _runners:
        _runners[S] = _make_runner(_get_compiled(S))
    return _runners[S]


def _build(S):
    fp32 = mybir.dt.float32
    bf16 = mybir.dt.bfloat16
    f8 = mybir.dt.float8e3
    nc = bacc.Bacc("TRN2", target_bir_lowering=False, debug=False,
                   num_devices=N_CORES)
    # features with hw on partitions: feat[u, q, s, c] = x[samp_s, c, u*98+q]
    feat = nc.dram_tensor("feat", [UC, HW1, S, C], f8, kind="ExternalInput")
    maskq = nc.dram_tensor("maskq", [HW1, UC, S], bf16, kind="ExternalInput")
    # weights: one 208-col pair (answer cols 0:208 = full chunk + ragged 80,
    # 832B lines at full DMA rate — no pad bytes) + 14 full 128-col chunks
    wtp = nc.dram_tensor("wtp", [E, 128, J, 208], f8, kind="ExternalInput")
    wt = nc.dram_tensor("wt", [NCH - 2, E, 128, J, 128], f8,
                        kind="ExternalInput")
    be = nc.dram_tensor("be", [E, N_ANS], bf16, kind="ExternalInput")
    ind = nc.dram_tensor("ind", [1, E, S], bf16, kind="ExternalInput")
    ind01 = nc.dram_tensor("ind01", [E, S], bf16, kind="ExternalInput")
    outT = nc.dram_tensor("outT", [N_ANS, S], bf16, kind="ExternalOutput")

    with tile.TileContext(nc) as tc:
        with (
            tc.tile_pool(name="persist", bufs=1) as persist,
            tc.tile_pool(name="fpool", bufs=2) as fpool,
            tc.tile_pool(name="wpool", bufs=10) as wpool,
            tc.tile_pool(name="psb", bufs=2, space="PSUM") as psb_pool,
            tc.tile_pool(name="ps2", bufs=4, space="PSUM") as ps2_pool,
        ):
            mask_sb = persist.tile([HW1, UC, S], bf16)
            indb = persist.tile([128, E, S], bf16)
            be_sb = persist.tile([E, N_ANS], bf16)
            i01_sb = persist.tile([E, S], bf16)

            # indicator-masked attended, bf16, for phase-2 streaming
            matt = persist.tile([128, E, S, J], bf16)
            out_sb = persist.tile([128, NCH, S], bf16)

            # phase 1: attT[c, s] = sum_hw feat[hw, c] * mask[hw, s] via PE
            # matvec chains (two 98-row matmuls per sample and c-chunk)
            first = True
            for i0 in range(0, S, GB):
                g_sz = min(GB, S - i0)
                ft = fpool.tile([HW1, UC, GB, C], f8, tag="ft")
                nc.sync.dma_start(
                    ft[:, :, :g_sz],
                    feat.ap()[:, :, i0:i0 + g_sz].rearrange(
                        "u q s c -> q u s c"))
                if first:
                    # small loads slot in right behind the first feature batch
                    nc.sync.dma_start(mask_sb[:], maskq.ap())
                    nc.sync.dma_start(indb[:],
                                      ind.ap().to_broadcast((128, E, S)))
                    nc.sync.dma_start(be_sb[:], be.ap())
                    nc.sync.dma_start(i01_sb[:], ind01.ap())
                    first = False
                ps_b = psb_pool.tile([128, GB, J], fp32, tag="psb")
                for s in range(g_sz):
                    for j in range(J):
                        for u in range(UC):
                            nc.tensor.matmul(
                                ps_b[:, s, j:j + 1],
                                ft[:, u, s, j * 128:(j + 1) * 128],
                                mask_sb[:, u, i0 + s:i0 + s + 1],
                                start=(u == 0), stop=(u == UC - 1))
                # fold the indicator (mean scale + weight descale) and cast
                # to bf16 for the phase-2 matmuls
                for g in range(E):
                    nc.vector.tensor_mul(
                        matt[:, g, i0:i0 + g_sz, :],
                        ps_b[:, :g_sz, :],
                        indb[:, g, i0:i0 + g_sz, None].to_broadcast(
                            (128, g_sz, J)))

            # phase 2, chunk-serial: psum[a, s] = be.T @ onehot
            #   + sum_g sum_j wt[g, j, a].T @ matt[:, g, :, j]
            # chunk table: (answer col offset, width, weight source)
            # chunks 0,1 come from the 208-wide pair DMA; 2..15 are singles
            for ac in range(NCH):
                if ac == 0:
                    wtile = wpool.tile([128, E, J, 208], f8, tag="wtp")
                    nc.sync.dma_start(
                        wtile[:], wtp.ap().rearrange("g p j a -> p g j a"))
                    pair = wtile
                    a0, aw, wsrc = 0, 128, pair[:, :, :, 0:128]
                elif ac == 1:
                    a0, aw, wsrc = 128, 80, pair[:, :, :, 128:208]
                else:
                    wtile = wpool.tile([128, E, J, 128], f8, tag="wt")
                    nc.sync.dma_start(
                        wtile[:],
                        wt.ap()[ac - 2].rearrange("g p j a -> p g j a"))
                    a0, aw, wsrc = 128 * ac - 48, 128, wtile
                ps = ps2_pool.tile([128, 512], fp32, tag="ps2")
                nc.tensor.matmul(
                    ps[:aw, :S],
                    be_sb[:, a0:a0 + aw],
                    i01_sb[:],
                    start=True, stop=False)
                for g in range(E):
                    for j in range(J):
                        nc.tensor.matmul(
                            ps[:aw, :S],
                            wsrc[:, g, j, :] if ac != 1 else pair[:, g, j, 128:208],
                            matt[:, g, :, j],
                            start=False,
                            stop=(g == E - 1 and j == J - 1))
                nc.vector.tensor_copy(out_sb[:aw, ac, :], ps[:aw, :S])

            # output DMAs: ragged pieces (chunks 0,1) early, then full-chunk
            # groups; the tiny final DMA drains right after chunk 15's copy.
            nc.sync.dma_start(outT.ap()[0:128], out_sb[:, 0, :])
            nc.sync.dma_start(outT.ap()[128:208], out_sb[:80, 1, :])
            for c0, c1 in ((2, 13), (13, 15), (15, 16)):
                nc.sync.dma_start(
                    outT.ap()[128 * c0 - 48:128 * c1 - 48].rearrange(
                        "(c p) s -> p c s", p=128),
                    out_sb[:, c0:c1, :])
    nc.compile()
    return nc


def _get_compiled(S):
    if S not in _compiled:
        _compiled[S] = _build(S)
    return _compiled[S]


def _exact_partition(cnt, cap):
    """Try to split the 32 experts into 8 groups of 4 with group-sum <= cap.

    Builds groups one at a time: each group takes the largest remaining
    expert plus 3 companions chosen by DFS over distinct count-combinations.
    Returns bins (list of expert-id groups) or None.
    """
    import itertools

    budget = [500000]

    def solve(ids):
        if not ids:
            return []
        if budget[0] <= 0:
            return None
        ids = sorted(ids, key=lambda e: -cnt[e])
        first = ids[0]
        rest = ids[1:]
        n = len(rest)
        seen = set()
        for combo in itertools.combinations(range(n), E - 1):
            budget[0] -= 1
            if budget[0] <= 0:
                return None
            vals = tuple(cnt[rest[i]] for i in combo)
            if cnt[first] + sum(vals) > cap or vals in seen:
                continue
            seen.add(vals)
            remaining = [rest[i] for i in range(n) if i not in combo]
            sub = solve(remaining)
            if sub is not None:
                return [[first] + [rest[i] for i in combo]] + sub
        return None

    return solve(list(range(N_EXP)))


def _route(instance):
    """Pack 32 experts into 8 bins of 4, balanced by sample count.

    Returns (bins, sample_lists, max_load): bins[c] = 4 expert ids,
    sample_lists[c] = sample indices routed to core c (grouped by expert).
    """
    cnt = np.bincount(instance, minlength=N_EXP)
    # perfect balance first: groups of 4 experts each with <= ceil(B/8)
    cap = (int(cnt.sum()) + N_CORES - 1) // N_CORES
    bins = _exact_partition(cnt, cap)
    if bins is None:
        order = np.argsort(-cnt, kind="stable")
        bins = [[] for _ in range(N_CORES)]
        loads = [0] * N_CORES
        for e in order:
            cands = [b for b in range(N_CORES) if len(bins[b]) < E]
            b = min(cands, key=lambda x: loads[x])
            bins[b].append(int(e))
            loads[b] += int(cnt[e])
    sample_lists = [
        np.concatenate([np.where(instance == e)[0] for e in bins[c]])
        for c in range(N_CORES)
    ]
    return bins, sample_lists, max(len(s) for s in sample_lists)


def make_in_maps(mask, features, weight, bias, inst, S, bins, sample_lists):
    feat_q = features.reshape(B, C, HWD).astype(F8E3)
    mask_flat = mask.reshape(B, HWD)
    # weights into e3m4 normal range; pad answer dim to A_PAD
    wq = (weight / W_DIV).astype(F8E3)
    in_maps = []
    for c in range(N_CORES):
        samp = sample_lists[c]
        n_c = len(samp)
        if n_c > 0:
            padded = np.concatenate([samp, np.full(S - n_c, samp[0])])
        else:
            padded = np.zeros(S, dtype=np.int64)
        ind_c = np.zeros((1, E, S), dtype=BF16)
        slot_of = {e: g for g, e in enumerate(bins[c])}
        for k in range(n_c):
            ind_c[0, slot_of[int(inst[samp[k]])], k] = K_IND
        be_c = np.ascontiguousarray(bias[bins[c]].astype(BF16))
        ind01_c = (np.asarray(ind_c[0], np.float32) != 0).astype(BF16)
        # [S,C,HWD] -> [HWD,S,C] -> [2,98,S,C] with hw = u*98+q
        feat_c = np.ascontiguousarray(
            feat_q[padded].transpose(2, 0, 1)).reshape(UC, HW1, S, C)
        # mask packed [q, u, s] so the DMA needs no rearrange
        mask_c = np.ascontiguousarray(
            mask_flat[padded].astype(BF16).T.reshape(UC, HW1, S)
            .transpose(1, 0, 2))
        wq_c = wq[bins[c]]  # [E, C, 2000]
        # pair: cols 0:208; singles: cols 208:2000 in 14 chunks of 128
        wtp_c = np.ascontiguousarray(
            wq_c[:, :, :208].reshape(E, J, 128, 208).transpose(0, 2, 1, 3))
        wt_c = np.ascontiguousarray(
            wq_c[:, :, 208:].reshape(E, J, 128, NCH - 2, 128)
            .transpose(3, 0, 2, 1, 4))
        in_maps.append({
            "feat": feat_c,
            "maskq": mask_c,
            "wtp": wtp_c,
            "wt": wt_c,
            "be": be_c,
            "ind": ind_c,
            "ind01": ind01_c,
        })
    return in_maps


def kernel(mask, features, weight, bias, instance):
    mask = np.ascontiguousarray(np.asarray(mask, dtype=np.float32))
    features = np.ascontiguousarray(np.asarray(features, dtype=np.float32))
    weight = np.ascontiguousarray(np.asarray(weight, dtype=np.float32))
    bias = np.ascontiguousarray(np.asarray(bias, dtype=np.float32))
    inst = np.asarray(instance).astype(np.int64)
    assert features.shape == (B, C, H, W)

    bins, sample_lists, max_load = _route(inst)
    S = max(S_DEFAULT, max_load)
    nc = _get_compiled(S)

    in_maps = make_in_maps(mask, features, weight, bias, inst, S, bins,
                           sample_lists)
    try:
        results = _get_runner(S)(in_maps)
    except Exception:
        results = run_bass_kernel_spmd(
            nc, in_maps, list(range(N_CORES))).results

    preds = np.empty((B, N_ANS), dtype=np.float32)
    for c in range(N_CORES):
        samp = sample_lists[c]
        preds[samp] = results[c]["outT"][:N_ANS, :len(samp)].astype(
            np.float32).T
    return preds


# Precompile the default-size program at import so a timed first call does
# not pay the (one-time) build+compile cost.
_get_compiled(S_DEFAULT)


# revision 47
# speedup vs baseline: 1.0424x; 1.0109x over previous
"""Trainium2 Bass kernel for masked-pool + per-sample expert matmul (moe_routing).

Computation (reference):
    attended[b,c] = mean_hw(mask[b,hw] * features[b,c,hw])        # [B,C]
    preds[b,a]    = sum_c attended[b,c] * weight[inst[b],c,a] + bias[inst[b],a]

Sharding: expert-parallel with host-side routing. The 32 experts are packed
into 8 bins of 4 (balanced by sample count); each core gets the features of
the samples routed to its 4 experts, its 4 experts' weights, and an indicator
matrix ind[slot, row] marking which rows belong to which expert slot.

All large operands are staged in fp8-e3m4 (4 mantissa bits, ~1.25% rms
quantization noise; the end-to-end rel err lands ~1.2e-2 vs the 2e-2 gate).
Weights are pre-scaled by ~128 so their magnitudes sit in e3m4's normal range
(max ~13.9 < 15.5); the descale rides along in the bf16 indicator constant
(~1/(196*128), carrying the spatial-mean divisor too), and the host weight
scale compensates that constant's bf16 rounding exactly. This halves HBM
traffic again vs bf16: ~7.6MB/core (~21us at 360GB/s).

Phase 1 (attended) runs on the PE as per-sample matvecs: features staged with
the spatial dim on partitions ([98x2] split of hw=196), so
attT[c, s] = sum_hw feat[hw, c] * mask[hw, s] is a chain of two 1-column
matmuls per (sample, c-chunk) accumulating in PSUM; a DVE multiply folds the
per-expert indicator (carrying mean + descale) into a bf16 matt[c, g, s, j].

Phase 2 streams the fp8 weights answer-chunk-serially: each 256KB weight DMA
covers one 128-col answer chunk for all 4 experts (512B contiguous lines);
right behind it the PE runs one bias matmul (K=4, bias arriving via
be.T @ onehot) plus 16 accumulate matmuls (expert x c-chunk) into a single
psum bank, then a DVE copy drains the chunk to bf16 SBUF. Accumulation
chains are strictly sequential per bank so no interleaved-chain hazards
exist, and the natural program order on the single SP DMA queue is already
the optimal serial transfer order (features -> weights -> outputs) with no
gating needed. Outputs drain in three groups (bulk / late pair / final
chunk) with 4 psum bufs — the shape that measured best against the Tile
scheduler's sem-wait coalescing — so only a single ~1.3us DMA-issue
pipeline plus a 56ns transfer trail the last weight tile.
"""

import numpy as np
import ml_dtypes

import concourse.bacc as bacc
import concourse.tile as tile
from concourse import mybir
from concourse.bass_utils import run_bass_kernel_spmd

BF16 = ml_dtypes.bfloat16
F8E3 = ml_dtypes.float8_e3m4

B, C, H, W = 256, 512, 14, 14
HWD = H * W  # 196
HW1 = HWD // 2  # 98 spatial positions per partition-chunk
UC = 2          # spatial chunks (2*98 = 196)
N_EXP, N_ANS = 32, 2000
N_CORES = 8
E = N_EXP // N_CORES  # expert slots per core = 4
S_DEFAULT = 32        # padded samples per core (>= max balanced bin load)
J = C // 128          # c-chunks = 4
A_PAD = 2048          # padded answer dim (2000 -> 16 chunks of 128)
NCH = A_PAD // 128    # 16 answer chunks of 128 cols = weight DMA granularity
GB = 16               # samples per feature-DMA batch
WSCALE = 128.0        # nominal weight pre-scale into e3m4 normal range
# the indicator is stored bf16; fold its rounding into the host weight scale
# so the device's matt * wq product is exactly w * att
K_IND = float(np.asarray(1.0 / (HWD * WSCALE), np.float32).astype(
    ml_dtypes.bfloat16))
W_DIV = float(np.float64(HWD) * np.float64(K_IND))  # wq = w / W_DIV

_compiled = {}  # S -> nc
_runners = {}   # S -> callable(in_maps) -> per-core result dicts


def _make_runner(nc):
    """Build a reusable jitted SPMD executor for `nc` (jit traced once, so
    repeat kernel() calls skip retracing; mirrors bass2jax.run_bass_via_pjrt).
    """
    import jax
    from jax.experimental.shard_map import shard_map
    from jax.sharding import Mesh, PartitionSpec
    from concourse.bass2jax import (_bass_exec_p, install_neuronx_cc_hook,
                                    partition_id_tensor)

    install_neuronx_cc_hook()
    pname = nc.partition_id_tensor.name if nc.partition_id_tensor else None
    in_names, out_names, out_avals = [], [], []
    for alloc in nc.m.functions[0].allocations:
        if not isinstance(alloc, mybir.MemoryLocationSet):
            continue
        name = alloc.memorylocations[0].name
        if alloc.kind == "ExternalInput":
            if name != pname:
                in_names.append(name)
        elif alloc.kind == "ExternalOutput":
            out_names.append(name)
            out_avals.append(jax.core.ShapedArray(
                tuple(alloc.tensor_shape), mybir.dt.np(alloc.dtype)))
    n_params = len(in_names)
    n_outs = len(out_avals)
    all_in = in_names + out_names + ([pname] if pname else [])
    donate = tuple(range(n_params, n_params + n_outs))

    def _body(*args):
        operands = list(args)
        if pname is not None:
            operands.append(partition_id_tensor())
        return tuple(_bass_exec_p.bind(
            *operands, out_avals=tuple(out_avals), in_names=tuple(all_in),
            out_names=tuple(out_names), lowering_input_output_aliases=(),
            sim_require_finite=True, sim_require_nnan=True, nc=nc))

    devices = jax.devices()[:N_CORES]
    mesh = Mesh(np.asarray(devices), ("core",))
    sharded = jax.jit(
        shard_map(_body, mesh=mesh,
                  in_specs=(PartitionSpec("core"),) * (n_params + n_outs),
                  out_specs=(PartitionSpec("core"),) * n_outs,
                  check_rep=False),
        donate_argnums=donate, keep_unused=True)

    def run(in_maps):
        concat_in = [
            np.concatenate([np.asarray(m[name]) for m in in_maps], axis=0)
            for name in in_names
        ]
        zeros = [np.zeros((N_CORES * a.shape[0], *a.shape[1:]), a.dtype)
                 for a in out_avals]
        out = sharded(*concat_in, *zeros)
        return [
            {name: np.asarray(out[i]).reshape(N_CORES, *out_avals[i].shape)[c]
             for i, name in enumerate(out_names)}
            for c in range(N_CORES)
        ]

    return run


def _get_runner(S):
    if S not in _runners:
        _runners[S] = _make_runner(_get_compiled(S))
    return _runners[S]


def _build(S):
    fp32 = mybir.dt.float32
    bf16 = mybir.dt.bfloat16
    f8 = mybir.dt.float8e3
    nc = bacc.Bacc("TRN2", target_bir_lowering=False, debug=False,
                   num_devices=N_CORES)
    # features with hw on partitions: feat[u, q, s, c] = x[samp_s, c, u*98+q]
    feat = nc.dram_tensor("feat", [UC, HW1, S, C], f8, kind="ExternalInput")
    maskq = nc.dram_tensor("maskq", [HW1, UC, S], bf16, kind="ExternalInput")
    # weights: one 208-col pair (answer cols 0:208 = full chunk + ragged 80,
    # 832B lines at full DMA rate — no pad bytes) + 14 full 128-col chunks
    wtp = nc.dram_tensor("wtp", [E, 128, J, 208], f8, kind="ExternalInput")
    wt = nc.dram_tensor("wt", [NCH - 2, E, 128, J, 128], f8,
                        kind="ExternalInput")
    be = nc.dram_tensor("be", [E, N_ANS], bf16, kind="ExternalInput")
    ind = nc.dram_tensor("ind", [1, E, S], bf16, kind="ExternalInput")
    ind01 = nc.dram_tensor("ind01", [E, S], bf16, kind="ExternalInput")
    outT = nc.dram_tensor("outT", [N_ANS, S], bf16, kind="ExternalOutput")

    with tile.TileContext(nc) as tc:
        with (
            tc.tile_pool(name="persist", bufs=1) as persist,
            tc.tile_pool(name="fpool", bufs=2) as fpool,
            tc.tile_pool(name="wpool", bufs=10) as wpool,
            tc.tile_pool(name="psb", bufs=2, space="PSUM") as psb_pool,
            tc.tile_pool(name="ps2", bufs=4, space="PSUM") as ps2_pool,
        ):
            mask_sb = persist.tile([HW1, UC, S], bf16)
            indb = persist.tile([128, E, S], bf16)
            ind_sb = persist.tile([1, E, S], bf16)
            ones_sb = persist.tile([1, 128], bf16)
            nc.vector.memset(ones_sb[:], 1.0)
            be_sb = persist.tile([E, N_ANS], bf16)
            i01_sb = persist.tile([E, S], bf16)

            # indicator-masked attended, bf16, for phase-2 streaming
            matt = persist.tile([128, E, S, J], bf16)
            out_sb = persist.tile([128, NCH, S], bf16)

            # phase 1: attT[c, s] = sum_hw feat[hw, c] * mask[hw, s] via PE
            # matvec chains (two 98-row matmuls per sample and c-chunk)
            first = True
            for i0 in range(0, S, GB):
                g_sz = min(GB, S - i0)
                ft = fpool.tile([HW1, UC, GB, C], f8, tag="ft")
                nc.sync.dma_start(
                    ft[:, :, :g_sz],
                    feat.ap()[:, :, i0:i0 + g_sz].rearrange(
                        "u q s c -> q u s c"))
                if first:
                    # small loads slot in right behind the first feature batch
                    nc.sync.dma_start(mask_sb[:], maskq.ap())
                    nc.sync.dma_start(ind_sb[:], ind.ap())
                    nc.sync.dma_start(be_sb[:], be.ap())
                    nc.sync.dma_start(i01_sb[:], ind01.ap())
                    # expand the [1, E, S] indicator across partitions via
                    # on-chip ones-matmuls instead of a 32KB broadcast DMA
                    ind_flat = ind_sb[:].rearrange("one e s -> one (e s)")
                    indb_flat = indb[:].rearrange("p e s -> p (e s)")
                    half = E * S // 2
                    for hh in range(2):
                        psbc = psb_pool.tile([128, GB, J], fp32, tag="psb")
                        pflat = psbc[:].rearrange("p g j -> p (g j)")
                        nc.tensor.matmul(
                            pflat[:, :half],
                            ones_sb[:],
                            ind_flat[:, hh * half:(hh + 1) * half],
                            start=True, stop=True)
                        nc.vector.tensor_copy(
                            indb_flat[:, hh * half:(hh + 1) * half],
                            pflat[:, :half])
                    first = False
                ps_b = psb_pool.tile([128, GB, J], fp32, tag="psb")
                for s in range(g_sz):
                    for j in range(J):
                        for u in range(UC):
                            nc.tensor.matmul(
                                ps_b[:, s, j:j + 1],
                                ft[:, u, s, j * 128:(j + 1) * 128],
                                mask_sb[:, u, i0 + s:i0 + s + 1],
                                start=(u == 0), stop=(u == UC - 1))
                # fold the indicator (mean scale + weight descale) and cast
                # to bf16 for the phase-2 matmuls
                for g in range(E):
                    nc.vector.tensor_mul(
                        matt[:, g, i0:i0 + g_sz, :],
                        ps_b[:, :g_sz, :],
                        indb[:, g, i0:i0 + g_sz, None].to_broadcast(
                            (128, g_sz, J)))

            # phase 2, chunk-serial: psum[a, s] = be.T @ onehot
            #   + sum_g sum_j wt[g, j, a].T @ matt[:, g, :, j]
            # chunk table: (answer col offset, width, weight source)
            # chunks 0,1 come from the 208-wide pair DMA; 2..15 are singles
            for ac in range(NCH):
                if ac == 0:
                    wtile = wpool.tile([128, E, J, 208], f8, tag="wtp")
                    nc.sync.dma_start(
                        wtile[:], wtp.ap().rearrange("g p j a -> p g j a"))
                    pair = wtile
                    a0, aw, wsrc = 0, 128, pair[:, :, :, 0:128]
                elif ac == 1:
                    a0, aw, wsrc = 128, 80, pair[:, :, :, 128:208]
                else:
                    wtile = wpool.tile([128, E, J, 128], f8, tag="wt")
                    nc.sync.dma_start(
                        wtile[:],
                        wt.ap()[ac - 2].rearrange("g p j a -> p g j a"))
                    a0, aw, wsrc = 128 * ac - 48, 128, wtile
                ps = ps2_pool.tile([128, 512], fp32, tag="ps2")
                nc.tensor.matmul(
                    ps[:aw, :S],
                    be_sb[:, a0:a0 + aw],
                    i01_sb[:],
                    start=True, stop=False)
                for g in range(E):
                    for j in range(J):
                        nc.tensor.matmul(
                            ps[:aw, :S],
                            wsrc[:, g, j, :] if ac != 1 else pair[:, g, j, 128:208],
                            matt[:, g, :, j],
                            start=False,
                            stop=(g == E - 1 and j == J - 1))
                nc.vector.tensor_copy(out_sb[:aw, ac, :], ps[:aw, :S])

            # output DMAs: ragged pieces (chunks 0,1) early, then full-chunk
            # groups; the tiny final DMA drains right after chunk 15's copy.
            nc.sync.dma_start(outT.ap()[0:128], out_sb[:, 0, :])
            nc.sync.dma_start(outT.ap()[128:208], out_sb[:80, 1, :])
            for c0, c1 in ((2, 13), (13, 15), (15, 16)):
                nc.sync.dma_start(
                    outT.ap()[128 * c0 - 48:128 * c1 - 48].rearrange(
                        "(c p) s -> p c s", p=128),
                    out_sb[:, c0:c1, :])
    nc.compile()
    return nc


def _get_compiled(S):
    if S not in _compiled:
        _compiled[S] = _build(S)
    return _compiled[S]


def _exact_partition(cnt, cap):
    """Try to split the 32 experts into 8 groups of 4 with group-sum <= cap.

    Builds groups one at a time: each group takes the largest remaining
    expert plus 3 companions chosen by DFS over distinct count-combinations.
    Returns bins (list of expert-id groups) or None.
    """
    import itertools

    budget = [500000]

    def solve(ids):
        if not ids:
            return []
        if budget[0] <= 0:
            return None
        ids = sorted(ids, key=lambda e: -cnt[e])
        first = ids[0]
        rest = ids[1:]
        n = len(rest)
        seen = set()
        for combo in itertools.combinations(range(n), E - 1):
            budget[0] -= 1
            if budget[0] <= 0:
                return None
            vals = tuple(cnt[rest[i]] for i in combo)
            if cnt[first] + sum(vals) > cap or vals in seen:
                continue
            seen.add(vals)
            remaining = [rest[i] for i in range(n) if i not in combo]
            sub = solve(remaining)
            if sub is not None:
                return [[first] + [rest[i] for i in combo]] + sub
        return None

    return solve(list(range(N_EXP)))


def _route(instance):
    """Pack 32 experts into 8 bins of 4, balanced by sample count.

    Returns (bins, sample_lists, max_load): bins[c] = 4 expert ids,
    sample_lists[c] = sample indices routed to core c (grouped by expert).
    """
    cnt = np.bincount(instance, minlength=N_EXP)
    # perfect balance first: groups of 4 experts each with <= ceil(B/8)
    cap = (int(cnt.sum()) + N_CORES - 1) // N_CORES
    bins = _exact_partition(cnt, cap)
    if bins is None:
        order = np.argsort(-cnt, kind="stable")
        bins = [[] for _ in range(N_CORES)]
        loads = [0] * N_CORES
        for e in order:
            cands = [b for b in range(N_CORES) if len(bins[b]) < E]
            b = min(cands, key=lambda x: loads[x])
            bins[b].append(int(e))
            loads[b] += int(cnt[e])
    sample_lists = [
        np.concatenate([np.where(instance == e)[0] for e in bins[c]])
        for c in range(N_CORES)
    ]
    return bins, sample_lists, max(len(s) for s in sample_lists)


def make_in_maps(mask, features, weight, bias, inst, S, bins, sample_lists):
    feat_q = features.reshape(B, C, HWD).astype(F8E3)
    mask_flat = mask.reshape(B, HWD)
    # weights into e3m4 normal range; pad answer dim to A_PAD
    wq = (weight / W_DIV).astype(F8E3)
    in_maps = []
    for c in range(N_CORES):
        samp = sample_lists[c]
        n_c = len(samp)
        if n_c > 0:
            padded = np.concatenate([samp, np.full(S - n_c, samp[0])])
        else:
            padded = np.zeros(S, dtype=np.int64)
        ind_c = np.zeros((1, E, S), dtype=BF16)
        slot_of = {e: g for g, e in enumerate(bins[c])}
        for k in range(n_c):
            ind_c[0, slot_of[int(inst[samp[k]])], k] = K_IND
        be_c = np.ascontiguousarray(bias[bins[c]].astype(BF16))
        ind01_c = (np.asarray(ind_c[0], np.float32) != 0).astype(BF16)
        # [S,C,HWD] -> [HWD,S,C] -> [2,98,S,C] with hw = u*98+q
        feat_c = np.ascontiguousarray(
            feat_q[padded].transpose(2, 0, 1)).reshape(UC, HW1, S, C)
        # mask packed [q, u, s] so the DMA needs no rearrange
        mask_c = np.ascontiguousarray(
            mask_flat[padded].astype(BF16).T.reshape(UC, HW1, S)
            .transpose(1, 0, 2))
        wq_c = wq[bins[c]]  # [E, C, 2000]
        # pair: cols 0:208; singles: cols 208:2000 in 14 chunks of 128
        wtp_c = np.ascontiguousarray(
            wq_c[:, :, :208].reshape(E, J, 128, 208).transpose(0, 2, 1, 3))
        wt_c = np.ascontiguousarray(
            wq_c[:, :, 208:].reshape(E, J, 128, NCH - 2, 128)
            .transpose(3, 0, 2, 1, 4))
        in_maps.append({
            "feat": feat_c,
            "maskq": mask_c,
            "wtp": wtp_c,
            "wt": wt_c,
            "be": be_c,
            "ind": ind_c,
            "ind01": ind01_c,
        })
    return in_maps


def kernel(mask, features, weight, bias, instance):
    mask = np.ascontiguousarray(np.asarray(mask, dtype=np.float32))
    features = np.ascontiguousarray(np.asarray(features, dtype=np.float32))
    weight = np.ascontiguousarray(np.asarray(weight, dtype=np.float32))
    bias = np.ascontiguousarray(np.asarray(bias, dtype=np.float32))
    inst = np.asarray(instance).astype(np.int64)
    assert features.shape == (B, C, H, W)

    bins, sample_lists, max_load = _route(inst)
    S = max(S_DEFAULT, max_load)
    nc = _get_compiled(S)

    in_maps = make_in_maps(mask, features, weight, bias, inst, S, bins,
                           sample_lists)
    try:
        results = _get_runner(S)(in_maps)
    except Exception:
        results = run_bass_kernel_spmd(
            nc, in_maps, list(range(N_CORES))).results

    preds = np.empty((B, N_ANS), dtype=np.float32)
    for c in range(N_CORES):
        samp = sample_lists[c]
        preds[samp] = results[c]["outT"][:N_ANS, :len(samp)].astype(
            np.float32).T
    return preds


# Precompile the default-size program at import so a timed first call does
# not pay the (one-time) build+compile cost.
_get_compiled(S_DEFAULT)
